# revision 14
# baseline (speedup 1.0000x reference)
"""Trainium2 Bass kernel for nn_ConvectionModule — low-rank formulation.

Math (reference):
    s = Z @ W_V                                   # [N]
    E = exp(sigmoid(s_i - s_j))                   # [N, N]
    out = (E / rowsum(E)) @ (Z @ W_C.T)           # [N, D]

E_ij = f(s_i - s_j) with f = exp o sigmoid, an analytic 1-D kernel, is
numerically low rank: f(u - v) ~= sum_k a_k(u) b_k(v) with b_0 == 1 and
K = 14 terms reaching ~1e-5 relative accuracy over the +-6 range that
covers s ~ N(0,1).  This collapses the O(N^2 D) attention into

    bz   = B @ Z            # [K, D]   (device: the only big reduction)
    rw   = bz @ W_C.T       # [K, D]
    out  = ACn @ rw         # [N, D]   ACn[i,k] = a_k(s_i) / denom_i

where denom_i = sum_k a_k(s_i) * (sum_j b_k(s_j)) is evaluated on the
host in float64 from the same quantized a/b tables the device uses
(host prep is O(N*K), same class as the baseline's host-computed s and
bias tables).  The b_k are re-orthogonalized (QR) over the actual s
sample so the K-channel sums carry no cancellation, which keeps every
bf16/fp8 rounding term ~2e-3 of the output.  Because b_0 == 1, the
dominant k=0 channel of bz is the plain column sum of Z, which the
host supplies exactly; the k>=1 channels are small corrections, so Z
streams to the device in fp8e3m4, halving the dominant DMA cost.

DMA plan (cost model: each HWDGE dma_start has a fixed ~625ns slot on
one shared sequencer): Z8 is pre-arranged partition-major on the host
and loaded in 4 big chunk DMAs (128 descriptors x 8KB each); all small
tensors are single DMAs; output chunks alternate between sync (HWDGE)
and gpsimd (SWDGE) queues.

Sharding: output rows are split across 8 cores (1024 each).  Every core
receives the full Z8/BT/WCT (replicated; cross-core collectives cost
>=15us here) plus its own 1024-row slice of ACn.
"""

import numpy as np

N = 8192
D = 512
NCORES = 8
M = N // NCORES            # 1024 output rows per core
P = 128
JT = N // P                # 64 j-tiles
K = 14                     # rank of the separable approximation
KB = K - 1                 # device-computed channels (k >= 1)
KS = 16                    # padded channel stride in psum_t
L = 6.0                    # fit domain [-L, L] for s
ZS = 32.0                  # Z8 upscale (keeps e4m3 operands out of subnormals)
GRID = 1601                # fit grid size
NCH = 4                    # Z8 chunk DMAs
TPC = JT // NCH            # tiles per chunk

_CACHE = {}


# --------------------------------------------------------------------------
# Rank-K separable fit of f(u - v) = exp(sigmoid(u - v)) with b_0 == 1.
# --------------------------------------------------------------------------

def _f(x):
    return np.exp(1.0 / (1.0 + np.exp(-np.asarray(x, dtype=np.float64))))


def _build_basis():
    g = np.linspace(-L, L, GRID)
    w = np.maximum(np.exp(-g * g / 2), 1e-4)
    w /= w.sum()
    F = _f(g[:, None] - g[None, :])
    a0 = F @ w                      # weighted projection onto b_0 == 1
    Gr = F - a0[:, None]
    su = np.sqrt(w)
    U, S, Vt = np.linalg.svd((su[:, None] * Gr) * su[None, :],
                             full_matrices=False)
    A = np.empty((GRID, K))
    B = np.empty((GRID, K))
    A[:, 0] = a0
    B[:, 0] = 1.0
    for k in range(1, K):
        A[:, k] = U[:, k - 1] * S[k - 1] / su
        B[:, k] = Vt[k - 1] / su
    return g, A, B


def _interp_cols(g, T, x):
    return np.stack([np.interp(x, g, T[:, k]) for k in range(T.shape[1])],
                    axis=1)


# --------------------------------------------------------------------------
# Kernel build
# --------------------------------------------------------------------------

def _build():
    import concourse.bass as bass  # noqa: F401
    import concourse.mybir as mybir
    import concourse.tile as tile
    from concourse import bacc
    from concourse.masks import make_identity

    f32 = mybir.dt.float32
    f32r = mybir.dt.float32r
    f16 = mybir.dt.float16
    bf16 = mybir.dt.bfloat16
    fp8 = mybir.dt.float8e4

    nc = bacc.Bacc("TRN2", target_bir_lowering=False, debug=False,
                   num_devices=NCORES)

    Z8 = nc.dram_tensor("Z8", [P, JT * D], fp8, kind="ExternalInput").ap()
    BT = nc.dram_tensor("BT", [P, JT * KB], fp8, kind="ExternalInput").ap()
    ACN = nc.dram_tensor("ACN", [K, M], f32r, kind="ExternalInput").ap()
    WCT = nc.dram_tensor("WCT", [P, 4 * D], f32r, kind="ExternalInput").ap()
    CS = nc.dram_tensor("CS", [P, 4], f32r, kind="ExternalInput").ap()
    Y = nc.dram_tensor("Y", [M, D], f16, kind="ExternalOutput").ap()

    with tile.TileContext(nc) as tc:
        with (
            tc.tile_pool(name="const", bufs=1) as constp,
            tc.tile_pool(name="zt", bufs=NCH) as ztp,
            tc.tile_pool(name="fin", bufs=4) as finp,
            tc.tile_pool(name="psW", bufs=2, space="PSUM") as psW,
            tc.tile_pool(name="psT", bufs=1, space="PSUM") as psT,
            tc.tile_pool(name="psR", bufs=1, space="PSUM") as psR,
            tc.tile_pool(name="psO", bufs=4, space="PSUM") as psO,
        ):
            # ---- identity + PE clock warm-up (overlaps input DMAs) --------
            id_b = constp.tile([P, P], bf16)
            make_identity(nc, id_b)
            dum = constp.tile([P, D], bf16)
            nc.vector.memset(dum[:], 0.0)
            actw = constp.tile([1, 2], bf16)
            nc.scalar.copy(actw[:], dum[0:1, 0:2])
            for wmm in range(14):
                wp = psW.tile([P, D], f32, tag="wp", name=f"wp{wmm}")
                nc.tensor.matmul(wp[:], id_b[:], dum[:],
                                 start=True, stop=True)

            # ---- inputs: CS+BT, then Z8 chunks, then WCT/ACN --------------
            cs = constp.tile([P, 4], f32r)
            nc.sync.dma_start(cs[:], CS)
            bt = constp.tile([P, JT, KB], fp8)
            nc.sync.dma_start(bt[:], BT.rearrange("p (t k) -> p t k", k=KB))
            psum_t = psT.tile([P, 4, KS], f32)
            zcs = []
            for g in range(NCH):
                zc = ztp.tile([P, TPC * D], fp8, tag="zc", name=f"zc{g}")
                nc.sync.dma_start(zc[:], Z8[:, g * TPC * D:(g + 1) * TPC * D])
                zcs.append(zc)
            wcts = []
            for dc in range(4):
                w = constp.tile([P, D], f32r, name=f"wct{dc}")
                nc.sync.dma_start(w[:], WCT[:, dc * D:(dc + 1) * D])
                wcts.append(w)
            acn = constp.tile([K, M], f32r)
            nc.sync.dma_start(acn[:], ACN)
            bzt = constp.tile([P, 4, K], f32r)
            nc.vector.tensor_copy(bzt[:, :, 0:1],
                                  cs[:].rearrange("p (c o) -> p c o", o=1))
            for g in range(NCH):
                zc = zcs[g]
                for tt in range(TPC):
                    t = g * TPC + tt
                    for dc in range(4):
                        nc.tensor.matmul(
                            psum_t[:, dc, 0:KB],
                            zc[:, tt * D + dc * P:tt * D + (dc + 1) * P],
                            bt[:, t, :],
                            start=(t == 0), stop=(t == JT - 1))
                nfill = (0, 0, 8, 2)[g]
                for wmm in range(nfill):
                    # keep the PE clock ramped through DMA / copy gaps
                    wp = psW.tile([P, D], f32, tag="wp",
                                  name=f"gf{g}_{wmm}")
                    nc.tensor.matmul(wp[:], id_b[:], dum[:],
                                     start=True, stop=True)

            # ---- assemble bzT (k>=1 from psum), phase 2: rw ---------------
            nc.vector.tensor_copy(bzt[:, :, 1:K], psum_t[:, :, 0:KB])
            psum_r = psR.tile([K, D], f32)
            for dc in range(4):
                nc.tensor.matmul(psum_r[:], bzt[:, dc, :], wcts[dc][:],
                                 start=(dc == 0), stop=(dc == 3))
            rw = constp.tile([K, D], f32r)
            nc.vector.tensor_copy(rw[:, 0:D // 2], psum_r[:, 0:D // 2])
            nc.vector.tensor_copy(rw[:, D // 2:D], psum_r[:, D // 2:D])
            for wmm in range(2):
                wp = psW.tile([P, D], f32, tag="wp", name=f"rwf{wmm}")
                nc.tensor.matmul(wp[:], id_b[:], dum[:], start=True,
                                 stop=True)

            # ---- phase 3: out chunk pairs -> fp16 -> DMA ------------------
            for pair in range(4):
                ysb = finp.tile([P, 2, D], f16, tag="ysb")
                for q in range(2):
                    c8 = pair * 2 + q
                    po = psO.tile([P, D], f32, tag="po")
                    nc.tensor.matmul(po[:], acn[:, c8 * P:(c8 + 1) * P],
                                     rw[:], start=True, stop=True)
                    if q == 0:
                        nc.vector.tensor_copy(ysb[:, q, :], po[:])
                    else:
                        nc.scalar.copy(ysb[:, q, :], po[:])
                nc.sync.dma_start(
                    Y[pair * 2 * P:(pair + 1) * 2 * P, :].rearrange(
                        "(q p) d -> p q d", p=P),
                    ysb[:])

    nc.compile()
    return nc


# --------------------------------------------------------------------------
# Host-side prep
# --------------------------------------------------------------------------

def make_in_maps(Z, W_C, W_V):
    import ml_dtypes

    fp8 = ml_dtypes.float8_e4m3
    bf16 = ml_dtypes.bfloat16

    Z = np.ascontiguousarray(Z, dtype=np.float32)
    W_C = np.ascontiguousarray(W_C, dtype=np.float32)
    W_V = np.ascontiguousarray(W_V, dtype=np.float32).reshape(D)

    if "basis" not in _CACHE:
        _CACHE["basis"] = _build_basis()
    g, A, B = _CACHE["basis"]

    s = Z.astype(np.float64) @ W_V.astype(np.float64)
    sc = np.clip(s, -L + 1e-6, L - 1e-6)
    a_raw = _interp_cols(g, A, sc)                 # [N, K] float64
    b_raw = _interp_cols(g, B, sc)                 # [N, K]

    # re-orthogonalize b over the empirical sample, keeping b_0 == 1
    Q, R = np.linalg.qr(b_raw)
    sgn = np.sign(np.diag(R))
    rt = np.sqrt(float(N))
    b = Q * sgn[None, :] * rt
    b[:, 0] = 1.0
    a = (a_raw @ R.T) * sgn[None, :] / rt

    for k in range(1, K):
        pw = 2.0 ** np.floor(np.log2(112.0 / np.abs(b[:, k]).max()))
        b[:, k] *= pw
        a[:, k] /= pw
    b_q = b.copy()
    b_q[:, 1:] = b[:, 1:].astype(fp8).astype(np.float64)
    t_sum = b_q.sum(axis=0)                        # [K] host, f64
    denom = a @ t_sum                              # [N]
    acn = (a / denom[:, None])
    acn[:, 1:] /= ZS                               # undo device Z8 scaling
    acn = acn.astype(np.float32)                   # [N, K]

    zt8 = (Z * ZS).astype(fp8)                     # [N, D], x32 (exact pow2)
    z8 = np.ascontiguousarray(                     # [P, JT*D] partition-major
        zt8.reshape(JT, P, D).transpose(1, 0, 2).reshape(P, JT * D))
    colsum = Z.astype(np.float64).sum(axis=0)      # [D] exact
    cs = np.ascontiguousarray(
        colsum.reshape(4, P).T.astype(np.float32))  # [P, 4]
    btv = np.ascontiguousarray(
        b_q[:, 1:].reshape(JT, P, KB).transpose(1, 0, 2)
        .reshape(P, JT * KB).astype(fp8))          # [P, JT*KB]
    wct = np.ascontiguousarray(                    # [P, 4*D] partition-major
        W_C.T.reshape(4, P, D).transpose(1, 0, 2).reshape(P, 4 * D)
        .astype(np.float32))

    in_maps = []
    for c in range(NCORES):
        acnT = np.ascontiguousarray(
            acn[c * M:(c + 1) * M].T)              # [K, M]
        in_maps.append({"Z8": z8, "BT": btv, "ACN": acnT,
                        "WCT": wct, "CS": cs})
    return in_maps


def kernel(Z, W_C, W_V):
    from concourse.bass_utils import run_bass_kernel_spmd

    if "nc" not in _CACHE:
        _CACHE["nc"] = _build()
    nc = _CACHE["nc"]

    in_maps = make_in_maps(Z, W_C, W_V)
    res = run_bass_kernel_spmd(nc, in_maps, core_ids=list(range(NCORES)))
    out = np.empty((N, D), dtype=np.float32)
    for c in range(NCORES):
        out[c * M:(c + 1) * M] = res.results[c]["Y"].astype(np.float32)
    return out
